# revision 23
# baseline (speedup 1.0000x reference)
"""GCN (gather -> segmented-scan aggregate -> transform -> pool) on 8 Trainium2
NeuronCores, data-parallel over graphs.

Math (reference GCNConv + mean-pool + linear + softmax):
    deg  = bincount(dst) + 1                     (self loops)
    dinv = 1/sqrt(deg)
    g    = dinv[:,None] * x                      (fold dinv[src] pre-gather)
    tot[n] = sum_{e: dst(e)=n} g[src(e)]         (gather + segmented scan)
    conv = (tot + g_local) * dinv[:,None] @ W1 + b1   (aggregate-then-transform)
    out  = softmax(mean_pool_per_graph(relu(conv)) @ W_out + b_out)

Sharding: graphs 64*c..64*(c+1) -> core c; edges partitioned by dst range and
dst-sorted; each core gathers g rows for its edges from a full replicated g.
Host does layout only (sort, CSR rowptr, index/mask arrays); all arithmetic on
device.
"""
import numpy as np

from concourse import bass, bacc, tile, mybir
from concourse.bass_utils import run_bass_kernel_spmd
from concourse.masks import make_identity

dt = mybir.dt

# hardcoded problem shape
N_NODES = 200000
N_EDGES = 4000000
N_GRAPHS = 512
D = 20
OUT = 5
NCORES = 8
P = 128
GPC = N_GRAPHS // NCORES         # graphs per core
SLOT6 = 6                        # node rows per PE-transpose batch in P2
OOB = np.int32(1 << 24)          # out-of-bounds sentinel for skipped gathers


# ---------------------------------------------------------------- host planner
def plan(x, edge_index, batch, W1, b1, W_out, b_out, n_graphs=N_GRAPHS,
         ncores=NCORES, kt=448):
    """Pure-layout host prep. Returns (cfg dict, list of per-core input maps)."""
    x = np.ascontiguousarray(np.asarray(x, np.float32))
    src = np.asarray(edge_index[0], np.int64)
    dst = np.asarray(edge_index[1], np.int64)
    batch = np.asarray(batch, np.int64)
    W1 = np.asarray(W1, np.float32)
    b1 = np.asarray(b1, np.float32).reshape(-1)
    W_out = np.asarray(W_out, np.float32)
    b_out = np.asarray(b_out, np.float32).reshape(-1)

    N = x.shape[0]
    E = src.shape[0]
    G = n_graphs
    gpc = G // ncores

    # padded node count: multiple of 128*4 for the 4-chunk P0 loop
    npp = -(-N // (P * 4)) * 4           # node rows per partition
    npad = P * npp

    # global dst-sort + CSR rowptr
    order = np.argsort(dst, kind="stable")
    srcs = src[order].astype(np.int32)
    cnt_node = np.bincount(dst, minlength=N)
    rowptr = np.zeros(N + 1, np.int64)
    np.cumsum(cnt_node, out=rowptr[1:])
    rowptr_pad = np.full(npad + 1, E, np.int64)
    rowptr_pad[: N + 1] = rowptr
    rowptr_i32 = rowptr_pad.astype(np.int32)

    # graph/core boundaries in node space (batch is sorted)
    gb = np.searchsorted(batch, np.arange(G + 1))     # [G+1]
    nb = gb[::gpc]                                    # [ncores+1]
    eb = rowptr[nb]

    # ---- per-core pass 1: partition cuts, sizes
    cores = []
    for c in range(ncores):
        n0, n1 = int(nb[c]), int(nb[c + 1])
        e0, e1 = int(eb[c]), int(eb[c + 1])
        Lc, Ec = n1 - n0, e1 - e0
        rl = rowptr[n0:n1 + 1] - e0                   # local CSR [Lc+1]
        tpp = -(-Ec // P) if Ec else 1
        cutn = np.searchsorted(rl, np.arange(P) * tpp, side="left")
        cutn = np.minimum(cutn, Lc)
        cutn[0] = 0
        cutn_full = np.append(cutn, Lc)
        cute = rl[cutn_full]                          # [P+1] edge cuts
        pplen = np.diff(cute)
        cores.append((n0, n1, e0, e1, Lc, Ec, rl, cutn_full, cute, pplen))

    epp_real = max(int(cc[9].max()) for cc in cores)
    nt1 = max(1, -(-epp_real // kt))
    epp = nt1 * kt
    k2_real = int(np.diff(gb).max())
    k2 = -(-k2_real // SLOT6) * SLOT6

    # ---- shared tensors
    x_pad = np.zeros((npad, D), np.float32)
    x_pad[:N] = x
    wblk = np.zeros((SLOT6 * D + 1, SLOT6 * D), np.float32)
    for s in range(SLOT6):
        wblk[s * D:(s + 1) * D, s * D:(s + 1) * D] = W1
    wblk[SLOT6 * D, :] = np.tile(b1, SLOT6)
    b1rep = np.broadcast_to(b1, (gpc, D)).copy()
    bout_rep = np.broadcast_to(b_out, (gpc, OUT)).copy()

    # ---- per-core pass 2: index/mask arrays
    in_maps = []
    for c in range(ncores):
        n0, n1, e0, e1, Lc, Ec, rl, cutn_full, cute, pplen = cores[c]
        src_arr = np.zeros((P, epp), np.int32)
        mask_arr = np.ones((P, epp), np.float32)
        p_of_e = np.repeat(np.arange(P), pplen)
        slot_of_e = np.arange(Ec) - np.repeat(cute[:-1], pplen)
        src_arr[p_of_e, slot_of_e] = srcs[e0:e1]
        nz = rl[:-1] < rl[1:]
        starts = rl[:-1][nz]                          # seg-start edge idxs
        mask_arr[p_of_e[starts], slot_of_e[starts]] = 0.0

        pn = np.searchsorted(cutn_full, np.arange(Lc), side="right") - 1
        lastslot = rl[1:] - 1 - cute[pn]
        extn = np.where(nz, pn * epp + lastslot, OOB).astype(np.int64)

        gbl = gb[c * gpc:(c + 1) * gpc + 1]
        gcnt = np.diff(gbl)
        q_of = np.repeat(np.arange(gpc), gcnt)
        k_of = np.arange(Lc) - np.repeat(gbl[:-1] - n0, gcnt)
        gat = np.full((gpc, k2), OOB, np.int32)
        ext2 = np.full((gpc, k2), OOB, np.int32)
        gat[q_of, k_of] = np.arange(n0, n1, dtype=np.int32)
        ext2[q_of, k_of] = extn.astype(np.int32)

        in_maps.append({
            "x_in": x_pad,
            "rowptr_in": rowptr_i32,
            "src_idx_in": src_arr,
            "mask_in": mask_arr,
            "ext_idx_in": ext2,
            "gat_idx_in": gat,
            "cnt_in": gcnt.astype(np.int32).reshape(gpc, 1),
            "wblk_in": wblk,
            "b1rep_in": b1rep,
            "wout_in": W_out,
            "bout_in": bout_rep,
        })

    cfg = dict(npp=npp, npad=npad, epp=epp, nt1=nt1, kt=kt, k2=k2, gpc=gpc)
    return cfg, in_maps


# ------------------------------------------------------------- device builder
def build_nc(cfg, dbg=False, repeat=1):
    npp, npad, epp, nt1, kt, k2, gpc = (
        cfg["npp"], cfg["npad"], cfg["epp"], cfg["nt1"], cfg["kt"],
        cfg["k2"], cfg["gpc"])
    f32, i32 = dt.float32, dt.int32
    mm, AL = mybir, mybir.AluOpType

    nc = bacc.Bacc("TRN2", target_bir_lowering=False, debug=False,
                   num_devices=NCORES)

    x_in = nc.dram_tensor("x_in", [npad, D], f32, kind="ExternalInput")
    rowptr_in = nc.dram_tensor("rowptr_in", [npad + 1], i32, kind="ExternalInput")
    src_idx_in = nc.dram_tensor("src_idx_in", [P, epp], i32, kind="ExternalInput")
    mask_in = nc.dram_tensor("mask_in", [P, epp], f32, kind="ExternalInput")
    ext_idx_in = nc.dram_tensor("ext_idx_in", [gpc, k2], i32, kind="ExternalInput")
    gat_idx_in = nc.dram_tensor("gat_idx_in", [gpc, k2], i32, kind="ExternalInput")
    cnt_in = nc.dram_tensor("cnt_in", [gpc, 1], i32, kind="ExternalInput")
    wblk_in = nc.dram_tensor("wblk_in", [SLOT6 * D + 1, SLOT6 * D], f32,
                             kind="ExternalInput")
    b1rep_in = nc.dram_tensor("b1rep_in", [gpc, D], f32, kind="ExternalInput")
    wout_in = nc.dram_tensor("wout_in", [D, OUT], f32, kind="ExternalInput")
    bout_in = nc.dram_tensor("bout_in", [gpc, OUT], f32, kind="ExternalInput")
    logits_out = nc.dram_tensor("logits_out", [gpc, OUT], f32,
                                kind="ExternalOutput")

    g_dram = nc.dram_tensor("g_scratch", [npad, D], f32)
    dinv_dram = nc.dram_tensor("dinv_scratch", [npad, 1], f32)
    ss_dram = nc.dram_tensor("ss_scratch", [P * epp, D], f32)

    dbg_t = {}
    if dbg:
        for nm, shp in [("dbg_dinv", [P, npp]), ("dbg_msg", [P, kt, D]),
                        ("dbg_scan", [P, kt, D]), ("dbg_tot", [gpc, k2, D]),
                        ("dbg_gloc", [gpc, k2, D]), ("dbg_dloc", [gpc, k2]),
                        ("dbg_conv", [gpc, k2, D]), ("dbg_relu", [gpc, k2, D]),
                        ("dbg_pool", [gpc, D])]:
            dbg_t[nm] = nc.dram_tensor(nm, shp, f32, kind="ExternalOutput")

    x_v = x_in.ap().rearrange("(p t) d -> p t d", p=P)        # [P, npp, D]
    g_v = g_dram.ap().rearrange("(p t) d -> p t d", p=P)
    dinv_v = dinv_dram.ap().rearrange("(p t) d -> p (t d)", p=P)   # [P, npp]
    ss_w = ss_dram.ap().rearrange("(p t) d -> p (t d)", p=P)  # [P, epp*D]

    NCH = 4
    CW = npp // NCH

    with tile.TileContext(nc) as tc:
        with tc.tile_pool(name="sb", bufs=2) as sb, \
             tc.tile_pool(name="sb1", bufs=1) as sb1, \
             tc.tile_pool(name="ps", bufs=2, space="PSUM") as ps:
          for _rep in range(repeat):
            # ---------------- P0: dinv + g = dinv*x -> DRAM
            rp_lo = sb1.tile([P, npp], i32)
            rp_hi = sb1.tile([P, npp], i32)
            nc.sync.dma_start(rp_lo[:], rowptr_in.ap()[0:npad]
                              .rearrange("(p t) -> p t", p=P))
            nc.sync.dma_start(rp_hi[:], rowptr_in.ap()[1:npad + 1]
                              .rearrange("(p t) -> p t", p=P))
            deg_t = sb1.tile([P, npp], f32)
            degi = sb1.tile([P, npp], i32)
            nc.vector.tensor_tensor(out=degi[:], in0=rp_hi[:], in1=rp_lo[:],
                                    op=AL.subtract)
            nc.vector.tensor_copy(deg_t[:], degi[:])          # int -> float
            nc.vector.tensor_scalar_add(deg_t[:], deg_t[:], 1.0)
            sq_t = sb1.tile([P, npp], f32)
            nc.scalar.activation(sq_t[:], deg_t[:],
                                 mm.ActivationFunctionType.Sqrt)
            dinv_t = sb1.tile([P, npp], f32)
            nc.vector.reciprocal(dinv_t[:], sq_t[:])
            nc.sync.dma_start(dinv_v, dinv_t[:])
            if dbg:
                nc.sync.dma_start(dbg_t["dbg_dinv"].ap(), dinv_t[:])

            for j in range(NCH):
                sl = slice(j * CW, (j + 1) * CW)
                x_t = sb.tile([P, CW, D], f32, tag="xg")
                nc.sync.dma_start(x_t[:], x_v[:, sl, :])
                gg_t = sb.tile([P, CW, D], f32, tag="gg")
                nc.vector.tensor_tensor(
                    out=gg_t[:], in0=x_t[:],
                    in1=dinv_t[:, sl].unsqueeze(2).to_broadcast([P, CW, D]),
                    op=AL.mult)
                nc.sync.dma_start(g_v[:, sl, :], gg_t[:])

            # ---------------- P1: gather + segmented scan + stream out
            prev_scan = None
            for t in range(nt1):
                esl = slice(t * kt, (t + 1) * kt)
                idx_t = sb.tile([P, kt], i32, tag="idx")
                nc.sync.dma_start(idx_t[:], src_idx_in.ap()[:, esl])
                msk_t = sb.tile([P, kt], f32, tag="msk")
                nc.sync.dma_start(msk_t[:], mask_in.ap()[:, esl])
                msg_t = sb.tile([P, kt, D], f32, tag="xg")
                # vector-DGE row gather: HW supports exactly one index per
                # partition per instruction (cf. klir_gather) -> kt instrs
                for k in range(kt):
                    nc.gpsimd.indirect_dma_start(
                        out=msg_t[:, k, :], out_offset=None, in_=g_dram.ap(),
                        in_offset=bass.IndirectOffsetOnAxis(
                            ap=idx_t[:, k:k + 1], axis=0))
                scan_t = sb.tile([P, kt, D], f32, tag="gg")
                for f in range(D):
                    init = (0.0 if t == 0
                            else prev_scan[:, kt - 1, f].unsqueeze(1))
                    nc.vector.tensor_tensor_scan(
                        out=scan_t[:, :, f], data0=msk_t[:],
                        data1=msg_t[:, :, f], initial=init,
                        op0=AL.mult, op1=AL.add)
                nc.sync.dma_start(
                    ss_w[:, t * kt * D:(t + 1) * kt * D],
                    scan_t[:].rearrange("p a b -> p (a b)"))
                if dbg and t == 0:
                    nc.sync.dma_start(dbg_t["dbg_msg"].ap(), msg_t[:])
                    nc.sync.dma_start(dbg_t["dbg_scan"].ap(), scan_t[:])
                prev_scan = scan_t

            # ---------------- P2: extract totals, transform, pool, logits
            # (big P2 tiles tag-share SBUF slots with the P1 tiles)
            tot_t = sb.tile([gpc, k2, D], f32, tag="xg")
            nc.vector.memset(tot_t[:], 0.0)
            eidx_t = sb.tile([gpc, k2], i32, tag="idx")
            nc.sync.dma_start(eidx_t[:], ext_idx_in.ap())
            for k in range(k2):
                nc.gpsimd.indirect_dma_start(
                    out=tot_t[:, k, :], out_offset=None, in_=ss_dram.ap(),
                    in_offset=bass.IndirectOffsetOnAxis(
                        ap=eidx_t[:, k:k + 1], axis=0),
                    bounds_check=P * epp - 1, oob_is_err=False)

            gloc_t = sb.tile([gpc, k2, D], f32, tag="gg")
            nc.vector.memset(gloc_t[:], 0.0)
            gidx_t = sb.tile([gpc, k2], i32, tag="msk")
            nc.sync.dma_start(gidx_t[:], gat_idx_in.ap())
            for k in range(k2):
                nc.gpsimd.indirect_dma_start(
                    out=gloc_t[:, k, :], out_offset=None, in_=g_dram.ap(),
                    in_offset=bass.IndirectOffsetOnAxis(
                        ap=gidx_t[:, k:k + 1], axis=0),
                    bounds_check=npad - 1, oob_is_err=False)

            dloc_t = sb.tile([gpc, k2], f32, tag="idx")
            nc.vector.memset(dloc_t[:], 0.0)
            nc.gpsimd.indirect_dma_start(
                out=dloc_t[:], out_offset=None, in_=dinv_dram.ap(),
                in_offset=bass.IndirectOffsetOnAxis(ap=gidx_t[:], axis=0),
                bounds_check=npad - 1, oob_is_err=False)

            if dbg:
                nc.sync.dma_start(dbg_t["dbg_tot"].ap(), tot_t[:])
                nc.sync.dma_start(dbg_t["dbg_gloc"].ap(), gloc_t[:])
                nc.sync.dma_start(dbg_t["dbg_dloc"].ap(), dloc_t[:])

            # conv_in = (tot + g_local) * dinv_local
            nc.vector.tensor_tensor(out=tot_t[:], in0=tot_t[:], in1=gloc_t[:],
                                    op=AL.add)
            nc.vector.tensor_tensor(
                out=tot_t[:], in0=tot_t[:],
                in1=dloc_t[:].unsqueeze(2).to_broadcast([gpc, k2, D]),
                op=AL.mult)

            if dbg:
                nc.sync.dma_start(dbg_t["dbg_conv"].ap(), tot_t[:])

            # transform: per 6 node-slots: PE transpose + blockdiag matmul
            ident = sb1.tile([P, P], f32)
            make_identity(nc, ident[:])
            wblk_t = sb1.tile([SLOT6 * D + 1, SLOT6 * D], f32)
            nc.sync.dma_start(wblk_t[:], wblk_in.ap())
            lhs_a = sb1.tile([SLOT6 * D + 1, gpc], f32, tag="lhs_a")
            lhs_b = sb1.tile([SLOT6 * D + 1, gpc], f32, tag="lhs_b")
            nc.vector.memset(lhs_a[:], 1.0)
            nc.vector.memset(lhs_b[:], 1.0)
            relu_s = sb.tile([gpc, k2, D], f32, tag="xg")

            W6 = SLOT6 * D
            for i6 in range(k2 // SLOT6):
                ssl = slice(i6 * SLOT6, (i6 + 1) * SLOT6)
                tr_ps = ps.tile([W6, gpc], f32, space="PSUM", tag="trp")
                nc.tensor.transpose(
                    out=tr_ps[:],
                    in_=tot_t[:, ssl, :].rearrange("p a b -> p (a b)"),
                    identity=ident[:gpc, :gpc])
                lhs_t = lhs_a if i6 % 2 == 0 else lhs_b
                nc.vector.tensor_copy(lhs_t[:W6, :], tr_ps[:])
                mm_ps = ps.tile([gpc, W6], f32, space="PSUM", tag="mmp")
                nc.tensor.matmul(mm_ps[:], lhsT=lhs_t[:], rhs=wblk_t[:],
                                 start=True, stop=True)
                nc.scalar.activation(
                    relu_s[:, ssl, :].rearrange("p a b -> p (a b)"),
                    mm_ps[:], mm.ActivationFunctionType.Relu)

            if dbg:
                nc.sync.dma_start(dbg_t["dbg_relu"].ap(), relu_s[:])

            # pooling: sum over node slots, pad-correct, divide by count
            pooled = sb1.tile([gpc, D], f32)
            nc.vector.tensor_reduce(
                out=pooled[:], in_=relu_s[:].transpose([0, 2, 1]),
                axis=mm.AxisListType.X, op=AL.add)

            cnt_t = sb1.tile([gpc, 1], i32)
            nc.sync.dma_start(cnt_t[:], cnt_in.ap())
            cntf = sb1.tile([gpc, 1], f32)
            nc.vector.tensor_copy(cntf[:], cnt_t[:])
            padn = sb1.tile([gpc, 1], f32)
            nc.vector.tensor_scalar(out=padn[:], in0=cntf[:], scalar1=-1.0,
                                    scalar2=float(k2), op0=AL.mult, op1=AL.add)
            b1r_t = sb1.tile([gpc, D], f32)
            nc.sync.dma_start(b1r_t[:], b1rep_in.ap())
            rb1 = sb1.tile([gpc, D], f32)
            nc.vector.tensor_scalar_max(rb1[:], b1r_t[:], 0.0)
            corr = sb1.tile([gpc, D], f32)
            nc.vector.tensor_tensor(out=corr[:], in0=rb1[:],
                                    in1=padn[:].to_broadcast([gpc, D]),
                                    op=AL.mult)
            nc.vector.tensor_tensor(out=pooled[:], in0=pooled[:], in1=corr[:],
                                    op=AL.subtract)
            cntm = sb1.tile([gpc, 1], f32)
            nc.vector.tensor_scalar_max(cntm[:], cntf[:], 1.0)
            rec = sb1.tile([gpc, 1], f32)
            nc.vector.reciprocal(rec[:], cntm[:])
            nc.vector.tensor_tensor(out=pooled[:], in0=pooled[:],
                                    in1=rec[:].to_broadcast([gpc, D]),
                                    op=AL.mult)

            if dbg:
                nc.sync.dma_start(dbg_t["dbg_pool"].ap(), pooled[:])

            # logits = pooled @ W_out + b_out ; softmax
            pt_ps = ps.tile([D, gpc], f32, space="PSUM", tag="ptp")
            nc.tensor.transpose(out=pt_ps[:], in_=pooled[:],
                                identity=ident[:gpc, :gpc])
            poolT = sb1.tile([D, gpc], f32)
            nc.vector.tensor_copy(poolT[:], pt_ps[:])
            wout_t = sb1.tile([D, OUT], f32)
            nc.sync.dma_start(wout_t[:], wout_in.ap())
            lg_ps = ps.tile([gpc, OUT], f32, space="PSUM", tag="lgp")
            nc.tensor.matmul(lg_ps[:], lhsT=poolT[:], rhs=wout_t[:],
                             start=True, stop=True)
            bout_t = sb1.tile([gpc, OUT], f32)
            nc.sync.dma_start(bout_t[:], bout_in.ap())
            lg_t = sb1.tile([gpc, OUT], f32)
            nc.vector.tensor_tensor(out=lg_t[:], in0=lg_ps[:], in1=bout_t[:],
                                    op=AL.add)
            mx_t = sb1.tile([gpc, 1], f32)
            nc.vector.tensor_reduce(out=mx_t[:], in_=lg_t[:],
                                    axis=mm.AxisListType.X, op=AL.max)
            nc.vector.tensor_tensor(out=lg_t[:], in0=lg_t[:],
                                    in1=mx_t[:].to_broadcast([gpc, OUT]),
                                    op=AL.subtract)
            ex_t = sb1.tile([gpc, OUT], f32)
            nc.scalar.activation(ex_t[:], lg_t[:],
                                 mm.ActivationFunctionType.Exp)
            sm_t = sb1.tile([gpc, 1], f32)
            nc.vector.tensor_reduce(out=sm_t[:], in_=ex_t[:],
                                    axis=mm.AxisListType.X, op=AL.add)
            rs_t = sb1.tile([gpc, 1], f32)
            nc.vector.reciprocal(rs_t[:], sm_t[:])
            out_t = sb1.tile([gpc, OUT], f32)
            nc.vector.tensor_tensor(out=out_t[:], in0=ex_t[:],
                                    in1=rs_t[:].to_broadcast([gpc, OUT]),
                                    op=AL.mult)
            nc.sync.dma_start(logits_out.ap(), out_t[:])

    nc.compile()
    return nc


# ------------------------------------------------------------------ entrypoint
_CACHE = {}


def kernel(x, edge_index, batch, W1, b1, W_out, b_out):
    cfg, in_maps = plan(x, edge_index, batch, W1, b1, W_out, b_out)
    key = tuple(sorted(cfg.items()))
    if key not in _CACHE:
        _CACHE[key] = build_nc(cfg)
    nc = _CACHE[key]
    res = run_bass_kernel_spmd(nc, in_maps, list(range(NCORES)))
    return np.concatenate([r["logits_out"] for r in res.results], axis=0)
